# revision 5
# baseline (speedup 1.0000x reference)
"""Bass/Trainium2 kernel for nn_Corr (mutual-softmax correlation + top-3).

Math:
  s = alpha * (pn^T @ qn)        pn/qn = channel-l2-normalized xp/xq, [256, 4096] per batch
  x_c = softmax_row(s) * softmax_col(s) = exp(2*s - ln rs_i - ln cs_j)
        with rs_i = sum_j exp(s_ij), cs_j = sum_i exp(s_ij)
  outputs: xc_o_q = x_c, xc_o_p = x_c^T (reshaped), valp/valq = top-3 along rows of
  x_c / x_c^T respectively.

Device strategy (8 NeuronCores = 4 batches x 2 row-halves):
  Launch 1 (stats): core (k,h) computes E = exp(alpha*s) for its 2048-row block,
    accumulating row sums (ScalarE activation accum_out) and column partials
    (VectorE adds + ones-matmul partition reduction).  Host combines halves and
    takes logs in float64.
  Launch 2 (output): core (k,h) recomputes its block of x_c AND of x_c^T (operands
    swapped; no on-chip transposes).  The per-column bias -ln cs_j/(2a) enters the
    matmul as an extra K=1 rank-1 channel (ones (x) chan vector); the per-row bias
    -ln rs_i is the ScalarE activation per-partition bias, so a single fused
    exp(2a*x + bias) pass from PSUM produces final output values.  Top-3 per row
    comes from the VectorE top-8 instruction (nc.vector.max).
  Matmuls run in float32r (FP22, full PE rate at N=512).
"""

import os

import numpy as np

B, C, H, W = 4, 256, 64, 64
N = H * W            # 4096 pixels
HALF = N // 2        # 2048 rows per core
P = 128              # partitions
NSTRIP = HALF // P   # 16 row strips per core
NCORES = 8

TRACE = bool(int(os.environ.get("CORR_TRACE", "0")))
LAST = {"stats_ns": None, "out_ns": None}

_programs = {}


def _build_stats(alpha):
    import concourse.bacc as bacc
    import concourse.mybir as mybir
    import concourse.tile as tile

    f32 = mybir.dt.float32
    f32r = mybir.dt.float32r
    AF = mybir.ActivationFunctionType

    nc = bacc.Bacc("TRN2", target_bir_lowering=False, debug=False, num_devices=NCORES)
    lhs = nc.dram_tensor("lhs", [C, HALF], f32, kind="ExternalInput").ap()
    rhs = nc.dram_tensor("rhs", [C, N], f32, kind="ExternalInput").ap()
    rowsum = nc.dram_tensor("rowsum", [P, NSTRIP], f32, kind="ExternalOutput").ap()
    colsum = nc.dram_tensor("colsum", [1, N], f32, kind="ExternalOutput").ap()

    with tile.TileContext(nc) as tc:
        with (
            tc.tile_pool(name="big", bufs=1) as big,
            tc.tile_pool(name="epool", bufs=3) as epool,
            tc.tile_pool(name="spool", bufs=2, space="PSUM") as spool,
            tc.tile_pool(name="small", bufs=1) as small,
        ):
            la = big.tile([P, HALF], f32r, tag="la")
            lb = big.tile([P, HALF], f32r, tag="lb")
            ra = big.tile([P, N], f32r, tag="ra")
            rb = big.tile([P, N], f32r, tag="rb")
            colacc = big.tile([P, N], f32, tag="colacc")
            rparts = small.tile([P, 2 * NSTRIP], f32, tag="rparts")
            rsum = small.tile([P, NSTRIP], f32, tag="rsum")
            ones128 = small.tile([P, P], f32, tag="ones128")
            cs_sb = small.tile([1, N], f32, tag="cs")

            nc.sync.dma_start(out=la, in_=lhs[0:P, :].bitcast(f32r))
            nc.sync.dma_start(out=lb, in_=lhs[P:C, :].bitcast(f32r))
            nc.sync.dma_start(out=ra, in_=rhs[0:P, :].bitcast(f32r))
            nc.sync.dma_start(out=rb, in_=rhs[P:C, :].bitcast(f32r))
            nc.vector.memset(colacc, 0.0)
            nc.vector.memset(ones128, 1.0)

            for s in range(NSTRIP):
                lha = la[:, s * P:(s + 1) * P]
                lhb = lb[:, s * P:(s + 1) * P]
                for h2 in range(2):
                    ps = spool.tile([P, HALF], f32, tag="ps")
                    for cb in range(4):
                        c0 = h2 * HALF + cb * 512
                        out_sl = ps[:, cb * 512:(cb + 1) * 512]
                        nc.tensor.matmul(out_sl, lha, ra[:, c0:c0 + 512],
                                         start=True, stop=False)
                        nc.tensor.matmul(out_sl, lhb, rb[:, c0:c0 + 512],
                                         start=False, stop=True)
                    et = epool.tile([P, HALF], f32, tag="et")
                    idx = s * 2 + h2
                    nc.scalar.activation(et, ps, AF.Exp, bias=0.0, scale=float(alpha),
                                         accum_out=rparts[:, idx:idx + 1])
                    acc_sl = colacc[:, h2 * HALF:(h2 + 1) * HALF]
                    nc.vector.tensor_add(acc_sl, acc_sl, et)

            nc.vector.reduce_sum(rsum, rparts.rearrange("p (s t) -> p s t", t=2),
                                 axis=mybir.AxisListType.X)
            nc.sync.dma_start(out=rowsum, in_=rsum)

            # column sums: ones^T @ colacc -> every output row holds the sums
            for h2 in range(2):
                ps2 = spool.tile([P, HALF], f32, tag="ps")
                for cb in range(4):
                    c0 = h2 * HALF + cb * 512
                    nc.tensor.matmul(ps2[:, cb * 512:(cb + 1) * 512],
                                     ones128, colacc[:, c0:c0 + 512],
                                     start=True, stop=True)
                nc.scalar.copy(cs_sb[0:1, h2 * HALF:(h2 + 1) * HALF], ps2[0:1, :])
            nc.sync.dma_start(out=colsum, in_=cs_sb)
    nc.compile()
    return nc


def _build_out(alpha):
    import concourse.bacc as bacc
    import concourse.mybir as mybir
    import concourse.tile as tile

    f32 = mybir.dt.float32
    f32r = mybir.dt.float32r
    AF = mybir.ActivationFunctionType

    nc = bacc.Bacc("TRN2", target_bir_lowering=False, debug=False, num_devices=NCORES)
    lhs_p = nc.dram_tensor("lhs_p", [C, HALF], f32, kind="ExternalInput").ap()
    rhs_q = nc.dram_tensor("rhs_q", [C, N], f32, kind="ExternalInput").ap()
    lhs_q = nc.dram_tensor("lhs_q", [C, HALF], f32, kind="ExternalInput").ap()
    rhs_p = nc.dram_tensor("rhs_p", [C, N], f32, kind="ExternalInput").ap()
    chan_q = nc.dram_tensor("chan_q", [1, N], f32, kind="ExternalInput").ap()
    chan_p = nc.dram_tensor("chan_p", [1, N], f32, kind="ExternalInput").ap()
    bias_r = nc.dram_tensor("bias_r", [P, NSTRIP], f32, kind="ExternalInput").ap()
    bias_c = nc.dram_tensor("bias_c", [P, NSTRIP], f32, kind="ExternalInput").ap()
    ones_in = nc.dram_tensor("ones_in", [1, P], f32, kind="ExternalInput").ap()
    xcq = nc.dram_tensor("xcq", [HALF, N], f32, kind="ExternalOutput").ap()
    xcp = nc.dram_tensor("xcp", [HALF, N], f32, kind="ExternalOutput").ap()
    tp8 = nc.dram_tensor("tp8", [NSTRIP, P, 8], f32, kind="ExternalOutput").ap()
    tq8 = nc.dram_tensor("tq8", [NSTRIP, P, 8], f32, kind="ExternalOutput").ap()

    with tile.TileContext(nc) as tc:
        with (
            tc.tile_pool(name="big", bufs=1) as big,
            tc.tile_pool(name="epool", bufs=2) as epool,
            tc.tile_pool(name="spool", bufs=2, space="PSUM") as spool,
            tc.tile_pool(name="small", bufs=1) as small,
            tc.tile_pool(name="t8pool", bufs=2) as t8pool,
        ):
            lpa = big.tile([P, HALF], f32r, tag="lpa")
            lpb = big.tile([P, HALF], f32r, tag="lpb")
            lqa = big.tile([P, HALF], f32r, tag="lqa")
            lqb = big.tile([P, HALF], f32r, tag="lqb")
            rqa = big.tile([P, N], f32r, tag="rqa")
            rqb = big.tile([P, N], f32r, tag="rqb")
            rpa = big.tile([P, N], f32r, tag="rpa")
            rpb = big.tile([P, N], f32r, tag="rpb")
            chq = small.tile([1, N], f32r, tag="chq")
            chp = small.tile([1, N], f32r, tag="chp")
            br = small.tile([P, NSTRIP], f32, tag="br")
            bc = small.tile([P, NSTRIP], f32, tag="bc")
            ones1 = small.tile([1, P], f32r, tag="ones1")

            nc.sync.dma_start(out=lpa, in_=lhs_p[0:P, :].bitcast(f32r))
            nc.sync.dma_start(out=lpb, in_=lhs_p[P:C, :].bitcast(f32r))
            nc.sync.dma_start(out=lqa, in_=lhs_q[0:P, :].bitcast(f32r))
            nc.sync.dma_start(out=lqb, in_=lhs_q[P:C, :].bitcast(f32r))
            nc.sync.dma_start(out=rqa, in_=rhs_q[0:P, :].bitcast(f32r))
            nc.sync.dma_start(out=rqb, in_=rhs_q[P:C, :].bitcast(f32r))
            nc.sync.dma_start(out=rpa, in_=rhs_p[0:P, :].bitcast(f32r))
            nc.sync.dma_start(out=rpb, in_=rhs_p[P:C, :].bitcast(f32r))
            nc.sync.dma_start(out=chq, in_=chan_q.bitcast(f32r))
            nc.sync.dma_start(out=chp, in_=chan_p.bitcast(f32r))
            nc.sync.dma_start(out=br, in_=bias_r)
            nc.sync.dma_start(out=bc, in_=bias_c)
            nc.sync.dma_start(out=ones1, in_=ones_in.bitcast(f32r))

            blocks = [
                (lpa, lpb, rqa, rqb, chq, br, xcq, tp8),
                (lqa, lqb, rpa, rpb, chp, bc, xcp, tq8),
            ]
            for (bla, blb, bra, brb, chan, bias, out_mat, out_top) in blocks:
                for s in range(NSTRIP):
                    lha = bla[:, s * P:(s + 1) * P]
                    lhb = blb[:, s * P:(s + 1) * P]
                    et = epool.tile([P, N], f32, tag="et")
                    for j in range(2):
                        ps = spool.tile([P, HALF], f32, tag="ps")
                        for cb in range(4):
                            c0 = j * HALF + cb * 512
                            out_sl = ps[:, cb * 512:(cb + 1) * 512]
                            nc.tensor.matmul(out_sl, lha,
                                             bra[:, c0:c0 + 512],
                                             start=True, stop=False)
                            nc.tensor.matmul(out_sl, lhb,
                                             brb[:, c0:c0 + 512],
                                             start=False, stop=False)
                            nc.tensor.matmul(out_sl, ones1,
                                             chan[0:1, c0:c0 + 512],
                                             start=False, stop=True)
                        nc.scalar.activation(et[:, j * HALF:(j + 1) * HALF], ps,
                                             AF.Exp, bias=bias[:, s:s + 1],
                                             scale=2.0 * float(alpha))
                    nc.sync.dma_start(out=out_mat[s * P:(s + 1) * P, :], in_=et)
                    t8 = t8pool.tile([P, 8], f32, tag="t8")
                    nc.vector.max(out=t8, in_=et)
                    nc.sync.dma_start(out=out_top[s], in_=t8)
    nc.compile()
    return nc


def _get_programs(alpha):
    key = round(float(alpha), 9)
    if key not in _programs:
        _programs[key] = (_build_stats(alpha), _build_out(alpha))
    return _programs[key]


def _run(nc, in_maps, tag):
    from concourse.bass_utils import run_bass_kernel_spmd

    core_ids = list(range(NCORES))
    if TRACE:
        try:
            res = run_bass_kernel_spmd(nc, in_maps, core_ids, trace=True)
            LAST[tag] = res.exec_time_ns
            return res.results
        except Exception as e:  # fall back to untraced execution
            print(f"[kernel] trace run failed ({e!r}); rerunning untraced")
    res = run_bass_kernel_spmd(nc, in_maps, core_ids)
    LAST[tag] = res.exec_time_ns
    return res.results


def kernel(xp, xq, alpha):
    alpha = float(np.asarray(alpha))
    xp = np.ascontiguousarray(np.asarray(xp, dtype=np.float32).reshape(B, C, N))
    xq = np.ascontiguousarray(np.asarray(xq, dtype=np.float32).reshape(B, C, N))

    # host: channel l2 normalization (fp32, matching the reference)
    def _norm(x):
        n = np.sqrt(np.einsum("kci,kci->ki", x, x, dtype=np.float32))
        return x * (1.0 / np.maximum(n, 1e-12))[:, None, :]

    pn = _norm(xp)
    qn = _norm(xq)

    nc_stats, nc_out = _get_programs(alpha)

    # ---- launch 1: stats ----
    in_maps = []
    for c in range(NCORES):
        k, h = divmod(c, 2)
        in_maps.append({
            "lhs": np.ascontiguousarray(pn[k][:, h * HALF:(h + 1) * HALF]),
            "rhs": np.ascontiguousarray(qn[k]),
        })
    res1 = _run(nc_stats, in_maps, "stats_ns")

    rs = np.empty((B, N), np.float64)
    cs = np.zeros((B, N), np.float64)
    for c in range(NCORES):
        k, h = divmod(c, 2)
        rs[k, h * HALF:(h + 1) * HALF] = \
            res1[c]["rowsum"].T.reshape(HALF).astype(np.float64)
        cs[k] += res1[c]["colsum"].reshape(N).astype(np.float64)
    lnrs = np.log(rs)
    lncs = np.log(cs)

    # ---- launch 2: outputs ----
    in_maps = []
    for c in range(NCORES):
        k, h = divmod(c, 2)
        sl = slice(h * HALF, (h + 1) * HALF)
        in_maps.append({
            "lhs_p": np.ascontiguousarray(pn[k][:, sl]),
            "rhs_q": np.ascontiguousarray(qn[k]),
            "lhs_q": np.ascontiguousarray(qn[k][:, sl]),
            "rhs_p": np.ascontiguousarray(pn[k]),
            "chan_q": (-lncs[k] / (2.0 * alpha)).astype(np.float32).reshape(1, N),
            "chan_p": (-lnrs[k] / (2.0 * alpha)).astype(np.float32).reshape(1, N),
            "bias_r": np.ascontiguousarray(
                (-lnrs[k, sl]).astype(np.float32).reshape(NSTRIP, P).T),
            "bias_c": np.ascontiguousarray(
                (-lncs[k, sl]).astype(np.float32).reshape(NSTRIP, P).T),
            "ones_in": np.ones((1, P), np.float32),
        })
    res2 = _run(nc_out, in_maps, "out_ns")

    xc_o_q = np.empty((B, N, H, W), np.float32)
    xc_o_p = np.empty((B, N, H, W), np.float32)
    valp = np.empty((B, 3, H, W), np.float32)
    valq = np.empty((B, 3, H, W), np.float32)
    for k in range(B):
        r0, r1 = res2[2 * k], res2[2 * k + 1]
        xc_o_q[k] = np.concatenate([r0["xcq"], r1["xcq"]], axis=0).reshape(N, H, W)
        xc_o_p[k] = np.concatenate([r0["xcp"], r1["xcp"]], axis=0).reshape(N, H, W)
        p8 = np.concatenate([r0["tp8"].reshape(HALF, 8),
                             r1["tp8"].reshape(HALF, 8)], axis=0)
        q8 = np.concatenate([r0["tq8"].reshape(HALF, 8),
                             r1["tq8"].reshape(HALF, 8)], axis=0)
        valp[k] = p8[:, :3].T.reshape(3, H, W)
        valq[k] = q8[:, :3].T.reshape(3, H, W)
    return (valp, valq, xc_o_p, xc_o_q)


# revision 9
# speedup vs baseline: 234.7663x; 234.7663x over previous
"""Bass/Trainium2 kernel for nn_Corr (mutual-softmax correlation + top-3).

Math:
  s = alpha * (pn^T @ qn)        pn/qn = channel-l2-normalized xp/xq, [256, 4096] per batch
  x_c = softmax_row(s) * softmax_col(s) = exp(2*s - ln rs_i - ln cs_j)
        with rs_i = sum_j exp(s_ij), cs_j = sum_i exp(s_ij)
  outputs: xc_o_q = x_c, xc_o_p = x_c^T (reshaped), valp/valq = top-3 along rows of
  x_c / x_c^T respectively.

Device strategy (8 NeuronCores = 4 batches x 2 row-halves):
  Launch 1 (stats): core (k,h) computes E = exp(alpha*s) for its 2048-row block,
    accumulating row sums (ScalarE activation accum_out) and column partials
    (VectorE adds + ones-matmul partition reduction).  Host combines halves and
    takes logs in float64.
  Launch 2 (output): core (k,h) recomputes its block of x_c AND of x_c^T (operands
    swapped; no on-chip transposes).  The per-column bias -ln cs_j/(2a) enters the
    matmul as an extra K=1 rank-1 channel (ones (x) chan vector); the per-row bias
    -ln rs_i is the ScalarE activation per-partition bias, so a single fused
    exp(2a*x + bias) pass from PSUM produces final output values.  Top-3 per row
    comes from the VectorE top-8 instruction (nc.vector.max).
  Matmuls run in float32r (FP22, full PE rate at N=512).
"""

import os

import numpy as np

B, C, H, W = 4, 256, 64, 64
N = H * W            # 4096 pixels
HALF = N // 2        # 2048 rows per core
P = 128              # partitions
NSTRIP = HALF // P   # 16 row strips per core
NCORES = 8

TRACE = bool(int(os.environ.get("CORR_TRACE", "0")))
LAST = {"stats_ns": None, "out_ns": None}

_programs = {}


def _build_stats(alpha):
    import concourse.bacc as bacc
    import concourse.mybir as mybir
    import concourse.tile as tile

    f32 = mybir.dt.float32
    f32r = mybir.dt.float32r
    AF = mybir.ActivationFunctionType

    nc = bacc.Bacc("TRN2", target_bir_lowering=False, debug=False, num_devices=NCORES)
    lhs = nc.dram_tensor("lhs", [C, HALF], f32, kind="ExternalInput").ap()
    rhs = nc.dram_tensor("rhs", [C, N], f32, kind="ExternalInput").ap()
    ones_in = nc.dram_tensor("ones_in", [P, P], f32, kind="ExternalInput").ap()
    rowsum = nc.dram_tensor("rowsum", [P, NSTRIP], f32, kind="ExternalOutput").ap()
    colsum = nc.dram_tensor("colsum", [1, N], f32, kind="ExternalOutput").ap()

    with tile.TileContext(nc) as tc:
        with (
            tc.tile_pool(name="big", bufs=1) as big,
            tc.tile_pool(name="epool", bufs=4) as epool,
            tc.tile_pool(name="spool", bufs=3, space="PSUM") as spool,
            tc.tile_pool(name="cpool", bufs=1, space="PSUM") as cpool,
            tc.tile_pool(name="small", bufs=1) as small,
        ):
            la = big.tile([P, HALF], f32r, tag="la")
            lb = big.tile([P, HALF], f32r, tag="lb")
            ra = big.tile([P, N], f32r, tag="ra")
            rb = big.tile([P, N], f32r, tag="rb")
            rparts = small.tile([P, 4 * NSTRIP], f32, tag="rparts")
            rsum = small.tile([P, NSTRIP], f32, tag="rsum")
            ones128 = small.tile([P, P], f32r, tag="ones128")
            cs_sb = small.tile([1, N], f32, tag="cs")

            nc.sync.dma_start(out=ones128, in_=ones_in.bitcast(f32r))
            for q in range(4):
                qs = slice(q * (HALF // 4), (q + 1) * (HALF // 4))
                nc.sync.dma_start(out=la[:, qs], in_=lhs[0:P, qs].bitcast(f32r))
                nc.sync.dma_start(out=lb[:, qs], in_=lhs[P:C, qs].bitcast(f32r))
            for q in range(4):
                qs = slice(q * (N // 4), (q + 1) * (N // 4))
                nc.sync.dma_start(out=ra[:, qs], in_=rhs[0:P, qs].bitcast(f32r))
                nc.sync.dma_start(out=rb[:, qs], in_=rhs[P:C, qs].bitcast(f32r))

            CW = 1024  # column super-chunk
            for J in range(N // CW):
                csps = cpool.tile([P, CW], f32, tag="csps")
                for s in range(NSTRIP):
                    lha = la[:, s * P:(s + 1) * P]
                    lhb = lb[:, s * P:(s + 1) * P]
                    ps = spool.tile([P, CW], f32, tag="ps")
                    for cb in range(CW // 512):
                        c0 = J * CW + cb * 512
                        out_sl = ps[:, cb * 512:(cb + 1) * 512]
                        nc.tensor.matmul(out_sl, lha, ra[:, c0:c0 + 512],
                                         start=True, stop=False)
                        nc.tensor.matmul(out_sl, lhb, rb[:, c0:c0 + 512],
                                         start=False, stop=True)
                    et = epool.tile([P, CW], f32r, tag="et")
                    nc.scalar.activation(et, ps, AF.Exp, bias=0.0, scale=float(alpha),
                                         accum_out=rparts[:, s * 4 + J:s * 4 + J + 1])
                    # column-sum accumulation on PE: csps += ones^T @ et
                    for cb in range(CW // 512):
                        nc.tensor.matmul(csps[:, cb * 512:(cb + 1) * 512],
                                         ones128,
                                         et[:, cb * 512:(cb + 1) * 512],
                                         start=(s == 0), stop=(s == NSTRIP - 1),
                                         skip_group_check=True)
                nc.scalar.copy(cs_sb[0:1, J * CW:(J + 1) * CW], csps[0:1, :])

            nc.vector.reduce_sum(rsum, rparts.rearrange("p (s t) -> p s t", t=4),
                                 axis=mybir.AxisListType.X)
            nc.sync.dma_start(out=rowsum, in_=rsum)
            nc.sync.dma_start(out=colsum, in_=cs_sb)
    nc.compile()
    return nc


def _build_out(alpha):
    import concourse.bacc as bacc
    import concourse.mybir as mybir
    import concourse.tile as tile

    f32 = mybir.dt.float32
    f32r = mybir.dt.float32r
    AF = mybir.ActivationFunctionType

    nc = bacc.Bacc("TRN2", target_bir_lowering=False, debug=False, num_devices=NCORES)
    lhs_p = nc.dram_tensor("lhs_p", [C, HALF], f32, kind="ExternalInput").ap()
    rhs_q = nc.dram_tensor("rhs_q", [C, N], f32, kind="ExternalInput").ap()
    lhs_q = nc.dram_tensor("lhs_q", [C, HALF], f32, kind="ExternalInput").ap()
    rhs_p = nc.dram_tensor("rhs_p", [C, N], f32, kind="ExternalInput").ap()
    chan_q = nc.dram_tensor("chan_q", [1, N], f32, kind="ExternalInput").ap()
    chan_p = nc.dram_tensor("chan_p", [1, N], f32, kind="ExternalInput").ap()
    bias_r = nc.dram_tensor("bias_r", [P, NSTRIP], f32, kind="ExternalInput").ap()
    bias_c = nc.dram_tensor("bias_c", [P, NSTRIP], f32, kind="ExternalInput").ap()
    ones_in = nc.dram_tensor("ones_in", [1, P], f32, kind="ExternalInput").ap()
    xcq = nc.dram_tensor("xcq", [HALF, N], f32, kind="ExternalOutput").ap()
    xcp = nc.dram_tensor("xcp", [HALF, N], f32, kind="ExternalOutput").ap()
    tp8 = nc.dram_tensor("tp8", [NSTRIP, P, 8], f32, kind="ExternalOutput").ap()
    tq8 = nc.dram_tensor("tq8", [NSTRIP, P, 8], f32, kind="ExternalOutput").ap()

    with tile.TileContext(nc) as tc:
        with (
            tc.tile_pool(name="big", bufs=1) as big,
            tc.tile_pool(name="epool", bufs=3) as epool,
            tc.tile_pool(name="spool", bufs=2, space="PSUM") as spool,
            tc.tile_pool(name="small", bufs=1) as small,
            tc.tile_pool(name="t8pool", bufs=4) as t8pool,
        ):
            lpa = big.tile([P, HALF], f32r, tag="lpa")
            lpb = big.tile([P, HALF], f32r, tag="lpb")
            lqa = big.tile([P, HALF], f32r, tag="lqa")
            lqb = big.tile([P, HALF], f32r, tag="lqb")
            rqa = big.tile([P, N], f32r, tag="rqa")
            rqb = big.tile([P, N], f32r, tag="rqb")
            rpa = big.tile([P, N], f32r, tag="rpa")
            rpb = big.tile([P, N], f32r, tag="rpb")
            chq = small.tile([1, N], f32r, tag="chq")
            chp = small.tile([1, N], f32r, tag="chp")
            br = small.tile([P, NSTRIP], f32, tag="br")
            bc = small.tile([P, NSTRIP], f32, tag="bc")
            ones1 = small.tile([1, P], f32r, tag="ones1")

            for q in range(4):
                qs = slice(q * (HALF // 4), (q + 1) * (HALF // 4))
                nc.sync.dma_start(out=lpa[:, qs], in_=lhs_p[0:P, qs].bitcast(f32r))
                nc.sync.dma_start(out=lpb[:, qs], in_=lhs_p[P:C, qs].bitcast(f32r))
                nc.sync.dma_start(out=lqa[:, qs], in_=lhs_q[0:P, qs].bitcast(f32r))
                nc.sync.dma_start(out=lqb[:, qs], in_=lhs_q[P:C, qs].bitcast(f32r))
            for q in range(4):
                qs = slice(q * (N // 4), (q + 1) * (N // 4))
                nc.sync.dma_start(out=rqa[:, qs], in_=rhs_q[0:P, qs].bitcast(f32r))
                nc.sync.dma_start(out=rqb[:, qs], in_=rhs_q[P:C, qs].bitcast(f32r))
                nc.sync.dma_start(out=rpa[:, qs], in_=rhs_p[0:P, qs].bitcast(f32r))
                nc.sync.dma_start(out=rpb[:, qs], in_=rhs_p[P:C, qs].bitcast(f32r))
            nc.sync.dma_start(out=chq, in_=chan_q.bitcast(f32r))
            nc.sync.dma_start(out=chp, in_=chan_p.bitcast(f32r))
            nc.sync.dma_start(out=br, in_=bias_r)
            nc.sync.dma_start(out=bc, in_=bias_c)
            nc.sync.dma_start(out=ones1, in_=ones_in.bitcast(f32r))

            blocks = [
                (lpa, lpb, rqa, rqb, chq, br, xcq, tp8),
                (lqa, lqb, rpa, rpb, chp, bc, xcp, tq8),
            ]
            for (bla, blb, bra, brb, chan, bias, out_mat, out_top) in blocks:
                for s in range(NSTRIP):
                    lha = bla[:, s * P:(s + 1) * P]
                    lhb = blb[:, s * P:(s + 1) * P]
                    et = epool.tile([P, N], f32, tag="et")
                    for j in range(2):
                        ps = spool.tile([P, HALF], f32, tag="ps")
                        for cb in range(4):
                            c0 = j * HALF + cb * 512
                            out_sl = ps[:, cb * 512:(cb + 1) * 512]
                            nc.tensor.matmul(out_sl, lha,
                                             bra[:, c0:c0 + 512],
                                             start=True, stop=False)
                            nc.tensor.matmul(out_sl, lhb,
                                             brb[:, c0:c0 + 512],
                                             start=False, stop=False)
                            nc.tensor.matmul(out_sl, ones1,
                                             chan[0:1, c0:c0 + 512],
                                             start=False, stop=True)
                        nc.scalar.activation(et[:, j * HALF:(j + 1) * HALF], ps,
                                             AF.Exp, bias=bias[:, s:s + 1],
                                             scale=2.0 * float(alpha))
                    nc.sync.dma_start(out=out_mat[s * P:(s + 1) * P, :], in_=et)
                    t8 = t8pool.tile([P, 8], f32, tag="t8")
                    nc.vector.max(out=t8, in_=et)
                    nc.sync.dma_start(out=out_top[s], in_=t8)
    nc.compile()
    return nc


def _get_programs(alpha):
    key = round(float(alpha), 9)
    if key not in _programs:
        _programs[key] = (_build_stats(alpha), _build_out(alpha))
    return _programs[key]


def _run(nc, in_maps, tag):
    from concourse.bass_utils import run_bass_kernel_spmd

    core_ids = list(range(NCORES))
    if TRACE:
        try:
            res = run_bass_kernel_spmd(nc, in_maps, core_ids, trace=True)
            LAST[tag] = res.exec_time_ns
            return res.results
        except Exception as e:  # fall back to untraced execution
            print(f"[kernel] trace run failed ({e!r}); rerunning untraced")
    res = run_bass_kernel_spmd(nc, in_maps, core_ids)
    LAST[tag] = res.exec_time_ns
    return res.results


def kernel(xp, xq, alpha):
    alpha = float(np.asarray(alpha))
    xp = np.ascontiguousarray(np.asarray(xp, dtype=np.float32).reshape(B, C, N))
    xq = np.ascontiguousarray(np.asarray(xq, dtype=np.float32).reshape(B, C, N))

    # host: channel l2 normalization (fp32, matching the reference)
    def _norm(x):
        n = np.sqrt(np.einsum("kci,kci->ki", x, x, dtype=np.float32))
        return x * (1.0 / np.maximum(n, 1e-12))[:, None, :]

    pn = _norm(xp)
    qn = _norm(xq)

    nc_stats, nc_out = _get_programs(alpha)

    # ---- launch 1: stats ----
    in_maps = []
    for c in range(NCORES):
        k, h = divmod(c, 2)
        in_maps.append({
            "lhs": np.ascontiguousarray(pn[k][:, h * HALF:(h + 1) * HALF]),
            "rhs": np.ascontiguousarray(qn[k]),
            "ones_in": np.ones((P, P), np.float32),
        })
    res1 = _run(nc_stats, in_maps, "stats_ns")

    rs = np.empty((B, N), np.float64)
    cs = np.zeros((B, N), np.float64)
    for c in range(NCORES):
        k, h = divmod(c, 2)
        rs[k, h * HALF:(h + 1) * HALF] = \
            res1[c]["rowsum"].T.reshape(HALF).astype(np.float64)
        cs[k] += res1[c]["colsum"].reshape(N).astype(np.float64)
    lnrs = np.log(rs)
    lncs = np.log(cs)

    # ---- launch 2: outputs ----
    in_maps = []
    for c in range(NCORES):
        k, h = divmod(c, 2)
        sl = slice(h * HALF, (h + 1) * HALF)
        in_maps.append({
            "lhs_p": np.ascontiguousarray(pn[k][:, sl]),
            "rhs_q": np.ascontiguousarray(qn[k]),
            "lhs_q": np.ascontiguousarray(qn[k][:, sl]),
            "rhs_p": np.ascontiguousarray(pn[k]),
            "chan_q": (-lncs[k] / (2.0 * alpha)).astype(np.float32).reshape(1, N),
            "chan_p": (-lnrs[k] / (2.0 * alpha)).astype(np.float32).reshape(1, N),
            "bias_r": np.ascontiguousarray(
                (-lnrs[k, sl]).astype(np.float32).reshape(NSTRIP, P).T),
            "bias_c": np.ascontiguousarray(
                (-lncs[k, sl]).astype(np.float32).reshape(NSTRIP, P).T),
            "ones_in": np.ones((1, P), np.float32),
        })
    res2 = _run(nc_out, in_maps, "out_ns")

    xc_o_q = np.empty((B, N, H, W), np.float32)
    xc_o_p = np.empty((B, N, H, W), np.float32)
    valp = np.empty((B, 3, H, W), np.float32)
    valq = np.empty((B, 3, H, W), np.float32)
    for k in range(B):
        r0, r1 = res2[2 * k], res2[2 * k + 1]
        xc_o_q[k] = np.concatenate([r0["xcq"], r1["xcq"]], axis=0).reshape(N, H, W)
        xc_o_p[k] = np.concatenate([r0["xcp"], r1["xcp"]], axis=0).reshape(N, H, W)
        p8 = np.concatenate([r0["tp8"].reshape(HALF, 8),
                             r1["tp8"].reshape(HALF, 8)], axis=0)
        q8 = np.concatenate([r0["tq8"].reshape(HALF, 8),
                             r1["tq8"].reshape(HALF, 8)], axis=0)
        valp[k] = p8[:, :3].T.reshape(3, H, W)
        valq[k] = q8[:, :3].T.reshape(3, H, W)
    return (valp, valq, xc_o_p, xc_o_q)


# revision 10
# speedup vs baseline: 577.4019x; 2.4595x over previous
"""Bass/Trainium2 kernel for nn_Corr (mutual-softmax correlation + top-3).

Math:
  s = alpha * (pn^T @ qn)        pn/qn = channel-l2-normalized xp/xq, [256, 4096] per batch
  x_c = softmax_row(s) * softmax_col(s) = exp(2*s - ln rs_i - ln cs_j)
        with rs_i = sum_j exp(s_ij), cs_j = sum_i exp(s_ij)
  outputs: xc_o_q = x_c, xc_o_p = x_c^T (reshaped), valp/valq = top-3 along rows of
  x_c / x_c^T respectively.

Device strategy (8 NeuronCores = 4 batches x 2 row-halves):
  Launch 1 (stats): core (k,h) computes E = exp(alpha*s) for its 2048-row block,
    accumulating row sums (ScalarE activation accum_out) and column partials
    (VectorE adds + ones-matmul partition reduction).  Host combines halves and
    takes logs in float64.
  Launch 2 (output): core (k,h) recomputes its block of x_c AND of x_c^T (operands
    swapped; no on-chip transposes).  The per-column bias -ln cs_j/(2a) enters the
    matmul as an extra K=1 rank-1 channel (ones (x) chan vector); the per-row bias
    -ln rs_i is the ScalarE activation per-partition bias, so a single fused
    exp(2a*x + bias) pass from PSUM produces final output values.  Top-3 per row
    comes from the VectorE top-8 instruction (nc.vector.max).
  Matmuls run in float32r (FP22, full PE rate at N=512).
"""

import os

import numpy as np

B, C, H, W = 4, 256, 64, 64
N = H * W            # 4096 pixels
HALF = N // 2        # 2048 rows per core
P = 128              # partitions
NSTRIP = HALF // P   # 16 row strips per core
NCORES = 8

TRACE = bool(int(os.environ.get("CORR_TRACE", "0")))
LAST = {"stats_ns": None, "out_ns": None}

_programs = {}


def _build_stats(alpha):
    import concourse.bacc as bacc
    import concourse.mybir as mybir
    import concourse.tile as tile

    f32 = mybir.dt.float32
    f32r = mybir.dt.float32r
    AF = mybir.ActivationFunctionType

    nc = bacc.Bacc("TRN2", target_bir_lowering=False, debug=False, num_devices=NCORES)
    lhs = nc.dram_tensor("lhs", [C, HALF], f32, kind="ExternalInput").ap()
    rhs = nc.dram_tensor("rhs", [C, N], f32, kind="ExternalInput").ap()
    ones_in = nc.dram_tensor("ones_in", [P, P], f32, kind="ExternalInput").ap()
    rowsum = nc.dram_tensor("rowsum", [P, NSTRIP], f32, kind="ExternalOutput").ap()
    colsum = nc.dram_tensor("colsum", [1, N], f32, kind="ExternalOutput").ap()

    with tile.TileContext(nc) as tc:
        with (
            tc.tile_pool(name="big", bufs=1) as big,
            tc.tile_pool(name="epool", bufs=4) as epool,
            tc.tile_pool(name="spool", bufs=3, space="PSUM") as spool,
            tc.tile_pool(name="cpool", bufs=1, space="PSUM") as cpool,
            tc.tile_pool(name="small", bufs=1) as small,
        ):
            la = big.tile([P, HALF], f32r, tag="la")
            lb = big.tile([P, HALF], f32r, tag="lb")
            ra = big.tile([P, N], f32r, tag="ra")
            rb = big.tile([P, N], f32r, tag="rb")
            rparts = small.tile([P, 4 * NSTRIP], f32, tag="rparts")
            rsum = small.tile([P, NSTRIP], f32, tag="rsum")
            ones128 = small.tile([P, P], f32r, tag="ones128")
            cs_sb = small.tile([1, N], f32, tag="cs")

            nc.sync.dma_start(out=ones128, in_=ones_in.bitcast(f32r))
            for q in range(4):
                qs = slice(q * (HALF // 4), (q + 1) * (HALF // 4))
                nc.sync.dma_start(out=la[:, qs], in_=lhs[0:P, qs].bitcast(f32r))
                nc.sync.dma_start(out=lb[:, qs], in_=lhs[P:C, qs].bitcast(f32r))
            for q in range(4):
                qs = slice(q * (N // 4), (q + 1) * (N // 4))
                nc.sync.dma_start(out=ra[:, qs], in_=rhs[0:P, qs].bitcast(f32r))
                nc.sync.dma_start(out=rb[:, qs], in_=rhs[P:C, qs].bitcast(f32r))

            CW = 1024  # column super-chunk
            for J in range(N // CW):
                csps = cpool.tile([P, CW], f32, tag="csps")
                for s in range(NSTRIP):
                    lha = la[:, s * P:(s + 1) * P]
                    lhb = lb[:, s * P:(s + 1) * P]
                    ps = spool.tile([P, CW], f32, tag="ps")
                    for cb in range(CW // 512):
                        c0 = J * CW + cb * 512
                        out_sl = ps[:, cb * 512:(cb + 1) * 512]
                        nc.tensor.matmul(out_sl, lha, ra[:, c0:c0 + 512],
                                         start=True, stop=False)
                        nc.tensor.matmul(out_sl, lhb, rb[:, c0:c0 + 512],
                                         start=False, stop=True)
                    et = epool.tile([P, CW], f32r, tag="et")
                    nc.scalar.activation(et, ps, AF.Exp, bias=0.0, scale=float(alpha),
                                         accum_out=rparts[:, s * 4 + J:s * 4 + J + 1])
                    # column-sum accumulation on PE: csps += ones^T @ et
                    for cb in range(CW // 512):
                        nc.tensor.matmul(csps[:, cb * 512:(cb + 1) * 512],
                                         ones128,
                                         et[:, cb * 512:(cb + 1) * 512],
                                         start=(s == 0), stop=(s == NSTRIP - 1),
                                         skip_group_check=True)
                nc.scalar.copy(cs_sb[0:1, J * CW:(J + 1) * CW], csps[0:1, :])

            nc.vector.reduce_sum(rsum, rparts.rearrange("p (s t) -> p s t", t=4),
                                 axis=mybir.AxisListType.X)
            nc.sync.dma_start(out=rowsum, in_=rsum)
            nc.sync.dma_start(out=colsum, in_=cs_sb)
    nc.compile()
    return nc


def _build_out(alpha):
    import concourse.bacc as bacc
    import concourse.mybir as mybir
    import concourse.tile as tile

    f32 = mybir.dt.float32
    f32r = mybir.dt.float32r
    AF = mybir.ActivationFunctionType

    nc = bacc.Bacc("TRN2", target_bir_lowering=False, debug=False, num_devices=NCORES)
    lhs_p = nc.dram_tensor("lhs_p", [C, HALF], f32, kind="ExternalInput").ap()
    rhs_q = nc.dram_tensor("rhs_q", [C, N], f32, kind="ExternalInput").ap()
    lhs_q = nc.dram_tensor("lhs_q", [C, HALF], f32, kind="ExternalInput").ap()
    rhs_p = nc.dram_tensor("rhs_p", [C, N], f32, kind="ExternalInput").ap()
    chan_q = nc.dram_tensor("chan_q", [1, N], f32, kind="ExternalInput").ap()
    chan_p = nc.dram_tensor("chan_p", [1, N], f32, kind="ExternalInput").ap()
    bias_r = nc.dram_tensor("bias_r", [P, NSTRIP], f32, kind="ExternalInput").ap()
    bias_c = nc.dram_tensor("bias_c", [P, NSTRIP], f32, kind="ExternalInput").ap()
    ones_in = nc.dram_tensor("ones_in", [1, P], f32, kind="ExternalInput").ap()
    xcq = nc.dram_tensor("xcq", [HALF, N], f32, kind="ExternalOutput").ap()
    xcp = nc.dram_tensor("xcp", [HALF, N], f32, kind="ExternalOutput").ap()
    tp8 = nc.dram_tensor("tp8", [NSTRIP, P, 8], f32, kind="ExternalOutput").ap()
    tq8 = nc.dram_tensor("tq8", [NSTRIP, P, 8], f32, kind="ExternalOutput").ap()

    with tile.TileContext(nc) as tc:
        with (
            tc.tile_pool(name="big", bufs=1) as big,
            tc.tile_pool(name="epool", bufs=3) as epool,
            tc.tile_pool(name="spool", bufs=2, space="PSUM") as spool,
            tc.tile_pool(name="small", bufs=1) as small,
            tc.tile_pool(name="t8pool", bufs=4) as t8pool,
        ):
            lpa = big.tile([P, HALF], f32r, tag="lpa")
            lpb = big.tile([P, HALF], f32r, tag="lpb")
            lqa = big.tile([P, HALF], f32r, tag="lqa")
            lqb = big.tile([P, HALF], f32r, tag="lqb")
            rqa = big.tile([P, N], f32r, tag="rqa")
            rqb = big.tile([P, N], f32r, tag="rqb")
            rpa = big.tile([P, N], f32r, tag="rpa")
            rpb = big.tile([P, N], f32r, tag="rpb")
            chq = small.tile([1, N], f32r, tag="chq")
            chp = small.tile([1, N], f32r, tag="chp")
            br = small.tile([P, NSTRIP], f32, tag="br")
            bc = small.tile([P, NSTRIP], f32, tag="bc")
            ones1 = small.tile([1, P], f32r, tag="ones1")

            for q in range(4):
                qs = slice(q * (HALF // 4), (q + 1) * (HALF // 4))
                nc.sync.dma_start(out=lpa[:, qs], in_=lhs_p[0:P, qs].bitcast(f32r))
                nc.sync.dma_start(out=lpb[:, qs], in_=lhs_p[P:C, qs].bitcast(f32r))
                nc.sync.dma_start(out=lqa[:, qs], in_=lhs_q[0:P, qs].bitcast(f32r))
                nc.sync.dma_start(out=lqb[:, qs], in_=lhs_q[P:C, qs].bitcast(f32r))
            for q in range(4):
                qs = slice(q * (N // 4), (q + 1) * (N // 4))
                nc.sync.dma_start(out=rqa[:, qs], in_=rhs_q[0:P, qs].bitcast(f32r))
                nc.sync.dma_start(out=rqb[:, qs], in_=rhs_q[P:C, qs].bitcast(f32r))
                nc.sync.dma_start(out=rpa[:, qs], in_=rhs_p[0:P, qs].bitcast(f32r))
                nc.sync.dma_start(out=rpb[:, qs], in_=rhs_p[P:C, qs].bitcast(f32r))
            nc.sync.dma_start(out=chq, in_=chan_q.bitcast(f32r))
            nc.sync.dma_start(out=chp, in_=chan_p.bitcast(f32r))
            nc.sync.dma_start(out=br, in_=bias_r)
            nc.sync.dma_start(out=bc, in_=bias_c)
            nc.sync.dma_start(out=ones1, in_=ones_in.bitcast(f32r))

            blocks = [
                (lpa, lpb, rqa, rqb, chq, br, xcq, tp8),
                (lqa, lqb, rpa, rpb, chp, bc, xcp, tq8),
            ]
            for (bla, blb, bra, brb, chan, bias, out_mat, out_top) in blocks:
                for s in range(NSTRIP):
                    lha = bla[:, s * P:(s + 1) * P]
                    lhb = blb[:, s * P:(s + 1) * P]
                    et = epool.tile([P, N], f32, tag="et")
                    for j in range(2):
                        ps = spool.tile([P, HALF], f32, tag="ps")
                        for cb in range(4):
                            c0 = j * HALF + cb * 512
                            out_sl = ps[:, cb * 512:(cb + 1) * 512]
                            nc.tensor.matmul(out_sl, lha,
                                             bra[:, c0:c0 + 512],
                                             start=True, stop=False)
                            nc.tensor.matmul(out_sl, lhb,
                                             brb[:, c0:c0 + 512],
                                             start=False, stop=False)
                            nc.tensor.matmul(out_sl, ones1,
                                             chan[0:1, c0:c0 + 512],
                                             start=False, stop=True)
                        nc.scalar.activation(et[:, j * HALF:(j + 1) * HALF], ps,
                                             AF.Exp, bias=bias[:, s:s + 1],
                                             scale=2.0 * float(alpha))
                    nc.sync.dma_start(out=out_mat[s * P:(s + 1) * P, 0:HALF],
                                      in_=et[:, 0:HALF])
                    nc.sync.dma_start(out=out_mat[s * P:(s + 1) * P, HALF:N],
                                      in_=et[:, HALF:N])
                    t8 = t8pool.tile([P, 8], f32, tag="t8")
                    nc.vector.max(out=t8, in_=et)
                    nc.sync.dma_start(out=out_top[s], in_=t8)
    nc.compile()
    return nc


def _get_programs(alpha):
    key = round(float(alpha), 9)
    if key not in _programs:
        _programs[key] = (_build_stats(alpha), _build_out(alpha))
    return _programs[key]


def _run(nc, in_maps, tag):
    from concourse.bass_utils import run_bass_kernel_spmd

    core_ids = list(range(NCORES))
    if TRACE:
        try:
            res = run_bass_kernel_spmd(nc, in_maps, core_ids, trace=True)
            LAST[tag] = res.exec_time_ns
            return res.results
        except Exception as e:  # fall back to untraced execution
            print(f"[kernel] trace run failed ({e!r}); rerunning untraced")
    res = run_bass_kernel_spmd(nc, in_maps, core_ids)
    LAST[tag] = res.exec_time_ns
    return res.results


def kernel(xp, xq, alpha):
    alpha = float(np.asarray(alpha))
    xp = np.ascontiguousarray(np.asarray(xp, dtype=np.float32).reshape(B, C, N))
    xq = np.ascontiguousarray(np.asarray(xq, dtype=np.float32).reshape(B, C, N))

    # host: channel l2 normalization (fp32, matching the reference)
    def _norm(x):
        n = np.sqrt(np.einsum("kci,kci->ki", x, x, dtype=np.float32))
        return x * (1.0 / np.maximum(n, 1e-12))[:, None, :]

    pn = _norm(xp)
    qn = _norm(xq)

    nc_stats, nc_out = _get_programs(alpha)

    # ---- launch 1: stats ----
    in_maps = []
    for c in range(NCORES):
        k, h = divmod(c, 2)
        in_maps.append({
            "lhs": np.ascontiguousarray(pn[k][:, h * HALF:(h + 1) * HALF]),
            "rhs": np.ascontiguousarray(qn[k]),
            "ones_in": np.ones((P, P), np.float32),
        })
    res1 = _run(nc_stats, in_maps, "stats_ns")

    rs = np.empty((B, N), np.float64)
    cs = np.zeros((B, N), np.float64)
    for c in range(NCORES):
        k, h = divmod(c, 2)
        rs[k, h * HALF:(h + 1) * HALF] = \
            res1[c]["rowsum"].T.reshape(HALF).astype(np.float64)
        cs[k] += res1[c]["colsum"].reshape(N).astype(np.float64)
    lnrs = np.log(rs)
    lncs = np.log(cs)

    # ---- launch 2: outputs ----
    in_maps = []
    for c in range(NCORES):
        k, h = divmod(c, 2)
        sl = slice(h * HALF, (h + 1) * HALF)
        in_maps.append({
            "lhs_p": np.ascontiguousarray(pn[k][:, sl]),
            "rhs_q": np.ascontiguousarray(qn[k]),
            "lhs_q": np.ascontiguousarray(qn[k][:, sl]),
            "rhs_p": np.ascontiguousarray(pn[k]),
            "chan_q": (-lncs[k] / (2.0 * alpha)).astype(np.float32).reshape(1, N),
            "chan_p": (-lnrs[k] / (2.0 * alpha)).astype(np.float32).reshape(1, N),
            "bias_r": np.ascontiguousarray(
                (-lnrs[k, sl]).astype(np.float32).reshape(NSTRIP, P).T),
            "bias_c": np.ascontiguousarray(
                (-lncs[k, sl]).astype(np.float32).reshape(NSTRIP, P).T),
            "ones_in": np.ones((1, P), np.float32),
        })
    res2 = _run(nc_out, in_maps, "out_ns")

    xc_o_q = np.empty((B, N, H, W), np.float32)
    xc_o_p = np.empty((B, N, H, W), np.float32)
    valp = np.empty((B, 3, H, W), np.float32)
    valq = np.empty((B, 3, H, W), np.float32)
    for k in range(B):
        r0, r1 = res2[2 * k], res2[2 * k + 1]
        xc_o_q[k] = np.concatenate([r0["xcq"], r1["xcq"]], axis=0).reshape(N, H, W)
        xc_o_p[k] = np.concatenate([r0["xcp"], r1["xcp"]], axis=0).reshape(N, H, W)
        p8 = np.concatenate([r0["tp8"].reshape(HALF, 8),
                             r1["tp8"].reshape(HALF, 8)], axis=0)
        q8 = np.concatenate([r0["tq8"].reshape(HALF, 8),
                             r1["tq8"].reshape(HALF, 8)], axis=0)
        valp[k] = p8[:, :3].T.reshape(3, H, W)
        valq[k] = q8[:, :3].T.reshape(3, H, W)
    return (valp, valq, xc_o_p, xc_o_q)
